# revision 14
# baseline (speedup 1.0000x reference)
"""MoE feed-forward (8 experts, top-2 routing) on 8 Trainium2 NeuronCores.

Strategy (balanced expert parallelism, all-bf16):
  - Router runs on host with jax-CPU, replicating the reference's fp32 ops
    (einsum + top_k + softmax) so expert selection matches exactly.
  - Expert identity is pure data under SPMD: every core runs the same
    program over two fixed-size token segments (s1=544, s2=512 slots), and
    each core's in_map supplies whichever experts' weights its segments
    need.  The 4 most-loaded experts are split across two cores' segment-A
    slots, the 4 least-loaded across two cores' segment-B slots, so every
    core processes C = s1+s2 = 1056 token slots (vs 1088 + phase-2 padding
    for one-expert-per-core).
  - The top-2 softmax combine weight is folded into the W2-path activations
    on the host (y = (silu(x@W1) * ((comb*x)@W2)) @ W3 is linear in the
    W2-path input), so the device applies no per-token scaling at all and
    phase 2 is token-granular.
  - Phase 1: h = silu(x@W1) * (xv@W2), bf16 in / f32 PSUM / bf16 h.
  - Phase 2: y[dslice] = W3_slice.T @ h contraction over d_ff with h as the
    moving operand; y stored bf16, host does the scatter-add combine.
"""

import sys
import types

for _p in ("/opt/trn_rl_repo", "/root/.axon_site/_ro/trn_rl_repo"):
    if _p not in sys.path:
        sys.path.append(_p)

import numpy as np
import ml_dtypes

import concourse.bass as bass
import concourse.mybir as mybir
import concourse.tile as tile
from concourse.bass_utils import run_bass_kernel_spmd

D_MODEL = 1024
D_FF = 4096
N_EXPERTS = 8
TOP_K = 2
P = 128
KO = D_MODEL // P  # 8 k-tiles over d_model
MF = D_FF // P  # 32 slices over d_ff
NQ = D_MODEL // P  # 8 output d_model slices
QD = P

F32 = mybir.dt.float32
BF16 = mybir.dt.bfloat16


# ---------------------------------------------------------------------------
# Workarounds for this container's toolchain
# ---------------------------------------------------------------------------
def _install_workarounds():
    # walrus here rejects >1 sync-wait on the TileContext-final Drain; split
    # the waits across a chain of single-wait drains.
    def _drain_and_barrier_split(self, tick_clock, wait_clock):
        drain_inst = self.nc.sync.drain()
        wait_clock.add_sem_waits(
            drain_inst.ins, tile.ScopedClock({None: tick_clock.global_clock})
        )
        si = drain_inst.ins.sync_info
        waits = list(si.on_wait) if si is not None else []
        if len(waits) > 1:
            si.on_wait = [waits[0]]
            for w in waits[1:]:
                d2 = self.nc.sync.drain()
                d2.ins.sync_info = mybir.SyncInfo(on_wait=[w], on_update=[])
        self.nc.all_engine_barrier()
        popped = self.nc._tile_sem_poison_stack.pop()
        assert popped is self._sem_poison
        self.nc.clear_and_free_semaphores(list(self.sems.allocated().values()))
        self.nc.all_engine_barrier()

    tile.TileContext._drain_and_barrier = _drain_and_barrier_split

    # antenv.axon_hooks is absent on this image; register the NTFF profile
    # hook from trn_agent_boot so trace=True works (no-op for trace=False).
    if "antenv.axon_hooks" not in sys.modules:
        try:
            from trn_agent_boot.trn_boot import _ntff_profile_via_ctypes

            hook = _ntff_profile_via_ctypes("/opt/axon/libaxon_pjrt.so")
        except Exception:
            hook = None
        mod = types.ModuleType("antenv.axon_hooks")
        mod.get_axon_ntff_profile_hook = lambda: hook
        mod.set_axon_ntff_profile_hook = lambda h: None
        sys.modules["antenv.axon_hooks"] = mod

    # artifact upload needs S3 creds we don't have; keep artifacts local.
    import concourse.bass_utils as bu

    bu.upload_artifacts = lambda tmpdir: "local://" + tmpdir

    # This walrus build accepts at most ONE sync-wait per non-DMA instruction
    # ("Too many sync wait commands"). Hoist extra waits onto single-wait
    # NoOps emitted just before the instruction on the same engine.
    import orjson

    def _split_multiwaits(bir: bytes) -> bytes:
        m = orjson.loads(bir)
        ctr = 0
        changed = False
        for f in m["functions"]:
            for blk in f["blocks"]:
                newinsts = []
                for inst in blk["instructions"]:
                    si = inst.get("sync_info")
                    if si and len(si.get("on_wait", [])) > 1:
                        waits = si["on_wait"]
                        for w in waits[:-1]:
                            ctr += 1
                            newinsts.append(
                                {
                                    "debug": inst.get("debug", 0),
                                    "engine": inst["engine"],
                                    "ins": [],
                                    "outs": [],
                                    "name": f"{inst['name']}_sw{ctr}",
                                    "opcode": "NoOp",
                                    "sync_info": {
                                        "on_wait": [w],
                                        "on_update": [],
                                    },
                                }
                            )
                        si["on_wait"] = [waits[-1]]
                        changed = True
                    newinsts.append(inst)
                blk["instructions"] = newinsts
        return orjson.dumps(m) if changed else bir

    _orig_tjb = bass.Bass.to_json_bytes

    def _to_json_bytes_split(self):
        return _split_multiwaits(_orig_tjb(self))

    bass.Bass.to_json_bytes = _to_json_bytes_split


_install_workarounds()


# ---------------------------------------------------------------------------
# Host-side router — replicates the reference router on jax-CPU
# ---------------------------------------------------------------------------
def _route(x, Wr, br):
    """Return comb [T, E] fp32 combine weights (0 for unselected experts) and
    top_idx [T, K] int — computed exactly as the reference does, on CPU."""
    import jax
    import jax.numpy as jnp

    cpu = jax.devices("cpu")[0]
    with jax.default_device(cpu):
        xj = jnp.asarray(np.asarray(x))
        logits = jnp.einsum("bsd,de->bse", xj, jnp.asarray(np.asarray(Wr)))
        logits = logits + jnp.asarray(np.asarray(br))
        top_vals, top_idx = jax.lax.top_k(logits, TOP_K)
        top_w = jax.nn.softmax(top_vals, axis=-1)
        comb = jnp.sum(
            jax.nn.one_hot(top_idx, N_EXPERTS, dtype=xj.dtype) * top_w[..., None],
            axis=-2,
        )
        comb_np = np.asarray(comb).reshape(-1, N_EXPERTS)
        idx_np = np.asarray(top_idx).reshape(-1, TOP_K)
    return comb_np, idx_np


def _seg_blocks(s):
    """Column blocks for a segment of s tokens. PSUM caps N at 512; an even
    split minimizes the per-instruction overhead (a 32-col remainder chain
    costs ~15ns/matmul extra, ~12us kernel-wide)."""
    nblk = -(-s // 512)
    base, rem = divmod(s, nblk)
    out = []
    t0 = 0
    for i in range(nblk):
        nb = base + (1 if i < rem else 0)
        out.append((t0, nb))
        t0 += nb
    return out


# ---------------------------------------------------------------------------
# Device program (two expert segments per core, SPMD)
# ---------------------------------------------------------------------------
_prog_cache = {}


def _build_program(s1, s2):
    """Bass program over C = s1 + s2 token slots: segment A = [0, s1) runs
    expert "a" weights, segment B = [s1, s1+s2) expert "b" weights.

    DMA sources are laid out for long contiguous runs (the DMA fabric moves
    packets per contiguous run; sub-1KB runs cost ~42ns each and cap well
    below line rate, >=4KB runs reach it).  x arrives as per-block packed
    chunks (u|v) sized to the phase-1 column blocks, streamed just-in-time
    for the first m-iteration; later m-iterations re-read the same tiles.
      xc<i> [P, 2, KO, nb] bf16   (j, ko, t) = x/xv[t0+t, ko*128+p]
      w120 [P, 2, KO, P] bf16     m=0 segment-A w1|w2 tile, 4KB runs
      w12a/b [MF, P, 2, KO, P]    (m, p, j, ko, f) = Wj[ko*128+p, m*128+f]
      w3a/b [NQ, P, MF, QD]       (q, p, k, d) = W3[k*128+p, q*128+d]
      y [NQ, P, C] bf16           (q, d, t) output, transposed layout
    """
    C = s1 + s2
    segs = [(0, s1), (s1, s2)]
    # (seg, global offset, width) per phase-1 block == per x chunk
    chunks = []
    for si, (off, s) in enumerate(segs):
        for t0, nb in _seg_blocks(s):
            chunks.append((si, off + t0, nb))

    nc = bass.Bass()
    xc = [
        nc.dram_tensor(f"xc{i}", [P, 2, KO, nb], BF16, kind="ExternalInput")
        for i, (_, _, nb) in enumerate(chunks)
    ]
    w120 = nc.dram_tensor("w120", [P, 2, KO, P], BF16, kind="ExternalInput")
    w12a = nc.dram_tensor("w12a", [MF, P, 2, KO, P], BF16, kind="ExternalInput")
    w12b = nc.dram_tensor("w12b", [MF, P, 2, KO, P], BF16, kind="ExternalInput")
    w3a = nc.dram_tensor("w3a", [NQ, P, MF, QD], BF16, kind="ExternalInput")
    w3b = nc.dram_tensor("w3b", [NQ, P, MF, QD], BF16, kind="ExternalInput")
    y = nc.dram_tensor("y", [NQ, P, C], BF16, kind="ExternalOutput")

    with tile.TileContext(nc) as tc:
        with (
            tc.tile_pool(name="persist", bufs=1) as persist,
            tc.tile_pool(name="wp", bufs=3) as wp,
            tc.tile_pool(name="w3p", bufs=2) as w3p,
            tc.tile_pool(name="sp", bufs=3) as sp,
            tc.tile_pool(name="yp", bufs=3) as yp,
            tc.tile_pool(name="psA", bufs=2, space="PSUM") as psA,
            tc.tile_pool(name="psB", bufs=2, space="PSUM") as psB,
            tc.tile_pool(name="psY", bufs=3, space="PSUM") as psY,
        ):
            h_sb = persist.tile([P, MF, C], BF16)
            w120_sb = persist.tile([P, 2, KO, P], BF16)
            xc_sb = []
            for i, (_, _, nb) in enumerate(chunks):
                t = persist.tile([P, 2, KO, nb], BF16, name=f"xc_sb{i}")
                xc_sb.append(t)

            # startup: w120 on sync; x chunks JIT on gpsimd/scalar.  The DMA
            # fabric round-robins packets with no priority, so chunk i+1's
            # descriptor is chained behind chunk i's arrival with a dummy
            # 8-element copy (WAR dep): the startup-critical first chunk gets
            # the full fabric, later chunks stream during m=0 compute.
            nc.sync.dma_start(w120_sb[:], w120[:])
            for i, t in enumerate(xc_sb):
                if i > 0:
                    nc.vector.tensor_copy(
                        t[:, 0, 0, 0:8], xc_sb[i - 1][:, 0, 0, 0:8]
                    )
                (nc.gpsimd if i % 2 == 0 else nc.scalar).dma_start(t[:], xc[i][:])

            w12_dr = {0: w12a, 1: w12b}

            prio_at_m = [None] * MF
            for m in range(MF):
                prio_at_m[m] = tc.cur_priority
                cur_si = -1
                w12t = None
                for ci, (si, off, nb) in enumerate(chunks):
                    if si != cur_si:
                        cur_si = si
                        if (m, si) == (0, 0):
                            w12t = w120_sb
                        else:
                            w12t = wp.tile(
                                [P, 2, KO, P], BF16, tag=f"w12t{si}", name="w12t"
                            )
                            if m in (1, 2):
                                nc.vector.tensor_copy(
                                    w12t[:, 0, 0, 0:8],
                                    xc_sb[min(m, len(xc_sb) - 1)][:, 0, 0, 0:8],
                                )
                            nc.sync.dma_start(w12t[:], w12_dr[si][m])
                    tsl = slice(off, off + nb)
                    ps1_f = psA.tile([P, nb], F32, tag="ps1", name="ps1",
                                     padded_shape=[P, 512])
                    ps2_f = psB.tile([P, nb], F32, tag="ps2", name="ps2",
                                     padded_shape=[P, 512])
                    for ko in range(KO):
                        nc.tensor.matmul(
                            ps1_f,
                            w12t[:, 0, ko],
                            xc_sb[ci][:, 0, ko],
                            start=(ko == 0),
                            stop=(ko == KO - 1),
                        )
                    for ko in range(KO):
                        nc.tensor.matmul(
                            ps2_f,
                            w12t[:, 1, ko],
                            xc_sb[ci][:, 1, ko],
                            start=(ko == 0),
                            stop=(ko == KO - 1),
                        )
                    sil = sp.tile([P, nb], F32, tag="sil", name="sil",
                                  padded_shape=[P, 512])
                    nc.scalar.activation(
                        sil, ps1_f, mybir.ActivationFunctionType.Silu
                    )
                    nc.vector.tensor_mul(h_sb[:, m, tsl], sil, ps2_f)

            # --- phase 2: y[q] = h.T @ W3[:, q*128:(q+1)*128], h moving ---
            w3_dr = {0: w3a, 1: w3b}
            for q in range(NQ):
                w3t = {}
                # prefetch this q's W3 as if issued mid-phase-1
                prio_save = tc.cur_priority
                if q < 2:
                    tc.cur_priority = prio_at_m[min(18 + 7 * q, MF - 1)]
                for si in range(2):
                    w3t[si] = w3p.tile(
                        [P, MF, QD], BF16, tag=f"w3t{si}", name=f"w3t{si}"
                    )
                    nc.sync.dma_start(w3t[si][:], w3_dr[si][q])
                tc.cur_priority = prio_save
                # biggest chains first, smallest last so the final store +
                # drain tail is as small as possible
                chains = []
                for si, (off, s) in enumerate(segs):
                    for t0, nb in _seg_blocks(s):
                        chains.append((si, off + t0, nb))
                chains.sort(key=lambda c: -c[2])
                for si, t0, nb in chains:
                    tsl = slice(t0, t0 + nb)
                    psy_f = psY.tile([P, nb], F32, tag="psy", name="psy",
                                     padded_shape=[P, 512])
                    for k in range(MF):
                        nc.tensor.matmul(
                            psy_f,
                            w3t[si][:, k],
                            h_sb[:, k, tsl],
                            start=(k == 0),
                            stop=(k == MF - 1),
                        )
                    ysb = yp.tile([P, nb], BF16, tag="ysb", name="ysb",
                                  padded_shape=[P, 512])
                    nc.vector.tensor_copy(ysb, psy_f)
                    nc.sync.dma_start(y[q, :, tsl], ysb)
    return nc


def _get_program(s1, s2):
    key = (s1, s2)
    if key not in _prog_cache:
        _prog_cache[key] = _build_program(s1, s2)
    return _prog_cache[key]


def _ceil64(n):
    return max(64, -(-n // 64) * 64)


# ---------------------------------------------------------------------------
# Public entry point
# ---------------------------------------------------------------------------
def kernel(x, Wr, br, W1, b1, W2, b2, W3, b3):
    x = np.asarray(x)
    Wr = np.asarray(Wr)
    br = np.asarray(br)
    W1 = np.asarray(W1)
    b1 = np.asarray(b1)
    W2 = np.asarray(W2)
    b2 = np.asarray(b2)
    W3 = np.asarray(W3)
    b3 = np.asarray(b3)

    B, S, _ = x.shape
    T = B * S
    xf = np.ascontiguousarray(x.reshape(T, D_MODEL))

    if np.any(b1) or np.any(b2):
        raise NotImplementedError("nonzero b1/b2 not supported by this kernel")

    comb, top_idx = _route(x, Wr, br)

    # Dispatch: gather each expert's tokens (host all-to-all).
    sels = []
    for e in range(N_EXPERTS):
        sel = np.nonzero((top_idx == e).any(axis=1))[0]
        sels.append(sel)
    counts = np.array([len(s) for s in sels])
    order = np.argsort(-counts, kind="stable")
    big, small = order[:4], order[4:]
    def _ceil16(n):
        return -(-n // 16) * 16

    s1 = max(512, _ceil16(-(-int(counts[big].max()) // 2)))
    s2 = max(256, _ceil16(-(-int(counts[small].max()) // 2)))
    C = s1 + s2

    # weight shuffles into DMA-friendly bf16 layouts (see _build_program)
    bf16 = ml_dtypes.bfloat16
    w1d = W1.reshape(N_EXPERTS, KO, P, MF, P).transpose(0, 3, 2, 1, 4).astype(bf16)
    w2d = W2.reshape(N_EXPERTS, KO, P, MF, P).transpose(0, 3, 2, 1, 4).astype(bf16)
    # pack w1|w2 per m-slice: [E, MF, P, 2, KO, P], 4KB contiguous rows
    w12d = np.ascontiguousarray(np.stack([w1d, w2d], axis=3))
    w3d = np.ascontiguousarray(
        W3.reshape(N_EXPERTS, MF, P, NQ, QD).transpose(0, 3, 2, 1, 4).astype(bf16)
    )

    # core 2i / 2i+1 share big[i] in segment A and small[i] in segment B
    seg_tok = {}  # core -> [(expert, tokens, off, size)]
    in_maps = []
    for c in range(8):
        i, half = divmod(c, 2)
        eb, es = int(big[i]), int(small[i])
        tokA = sels[eb][half * s1 : (half + 1) * s1]
        tokB = sels[es][half * s2 : (half + 1) * s2]
        seg_tok[c] = [(eb, tokA, 0), (es, tokB, s1)]

        xu_c = np.zeros((P, KO, C), dtype=bf16)
        xv_c = np.zeros((P, KO, C), dtype=bf16)
        for e, toks, off in seg_tok[c]:
            n = len(toks)
            if not n:
                continue
            xs = xf[toks]
            xu_c[:, :, off : off + n] = (
                xs.astype(bf16).reshape(n, KO, P).transpose(2, 1, 0)
            )
            xv_c[:, :, off : off + n] = (
                (xs * comb[toks, e][:, None])
                .astype(bf16)
                .reshape(n, KO, P)
                .transpose(2, 1, 0)
            )
        im = {
            "w120": w12d[eb][0],
            "w12a": w12d[eb],
            "w12b": w12d[es],
            "w3a": w3d[eb],
            "w3b": w3d[es],
        }
        ci = 0
        for soff, s in ((0, s1), (s1, s2)):
            for t0, nb in _seg_blocks(s):
                a, b = soff + t0, soff + t0 + nb
                im[f"xc{ci}"] = np.ascontiguousarray(
                    np.stack([xu_c[:, :, a:b], xv_c[:, :, a:b]], axis=1)
                )
                ci += 1
        in_maps.append(im)

    nc = _get_program(s1, s2)
    try:
        res = run_bass_kernel_spmd(nc, in_maps, core_ids=list(range(N_EXPERTS)))
    except Exception:
        # transient NRT/axon device hiccups have been observed; retry once
        import time as _time

        _time.sleep(5)
        res = run_bass_kernel_spmd(nc, in_maps, core_ids=list(range(N_EXPERTS)))

    # Combine: scatter-add expert outputs (softmax weights already folded in).
    out = np.zeros((T, D_MODEL), dtype=np.float32)
    for c in range(8):
        yc = np.asarray(res.results[c]["y"], dtype=np.float32)  # [NQ, P, C]
        yt = yc.transpose(2, 0, 1).reshape(C, D_MODEL)
        for e, toks, off in seg_tok[c]:
            n = len(toks)
            if n:
                out[toks] += yt[off : off + n]
    if np.any(b3):
        out += comb @ b3
    return out.reshape(B, S, D_MODEL)


# revision 24
# speedup vs baseline: 1.0264x; 1.0264x over previous
"""MoE feed-forward (8 experts, top-2 routing) on 8 Trainium2 NeuronCores.

Strategy (balanced expert parallelism, all-bf16):
  - Router runs on host with jax-CPU, replicating the reference's fp32 ops
    (einsum + top_k + softmax) so expert selection matches exactly.
  - Expert identity is pure data under SPMD: every core runs the same
    program over two fixed-size token segments (s1=544, s2=512 slots), and
    each core's in_map supplies whichever experts' weights its segments
    need.  The 4 most-loaded experts are split across two cores' segment-A
    slots, the 4 least-loaded across two cores' segment-B slots, so every
    core processes C = s1+s2 = 1056 token slots (vs 1088 + phase-2 padding
    for one-expert-per-core).
  - The top-2 softmax combine weight is folded into the W2-path activations
    on the host (y = (silu(x@W1) * ((comb*x)@W2)) @ W3 is linear in the
    W2-path input), so the device applies no per-token scaling at all and
    phase 2 is token-granular.
  - Phase 1: h = silu(x@W1) * (xv@W2), bf16 in / f32 PSUM / bf16 h.
  - Phase 2: y[dslice] = W3_slice.T @ h contraction over d_ff with h as the
    moving operand; y stored bf16, host does the scatter-add combine.
"""

import sys
import types

for _p in ("/opt/trn_rl_repo", "/root/.axon_site/_ro/trn_rl_repo"):
    if _p not in sys.path:
        sys.path.append(_p)

import numpy as np
import ml_dtypes

import concourse.bass as bass
import concourse.mybir as mybir
import concourse.tile as tile
from concourse.bass_utils import run_bass_kernel_spmd

D_MODEL = 1024
D_FF = 4096
N_EXPERTS = 8
TOP_K = 2
P = 128
KO = D_MODEL // P  # 8 k-tiles over d_model
MF = D_FF // P  # 32 slices over d_ff
NQ = D_MODEL // P  # 8 output d_model slices
QD = P

F32 = mybir.dt.float32
BF16 = mybir.dt.bfloat16


# ---------------------------------------------------------------------------
# Workarounds for this container's toolchain
# ---------------------------------------------------------------------------
def _install_workarounds():
    # walrus here rejects >1 sync-wait on the TileContext-final Drain; split
    # the waits across a chain of single-wait drains.
    def _drain_and_barrier_split(self, tick_clock, wait_clock):
        drain_inst = self.nc.sync.drain()
        wait_clock.add_sem_waits(
            drain_inst.ins, tile.ScopedClock({None: tick_clock.global_clock})
        )
        si = drain_inst.ins.sync_info
        waits = list(si.on_wait) if si is not None else []
        if len(waits) > 1:
            si.on_wait = [waits[0]]
            for w in waits[1:]:
                d2 = self.nc.sync.drain()
                d2.ins.sync_info = mybir.SyncInfo(on_wait=[w], on_update=[])
        self.nc.all_engine_barrier()
        popped = self.nc._tile_sem_poison_stack.pop()
        assert popped is self._sem_poison
        self.nc.clear_and_free_semaphores(list(self.sems.allocated().values()))
        self.nc.all_engine_barrier()

    tile.TileContext._drain_and_barrier = _drain_and_barrier_split

    # antenv.axon_hooks is absent on this image; register the NTFF profile
    # hook from trn_agent_boot so trace=True works (no-op for trace=False).
    if "antenv.axon_hooks" not in sys.modules:
        try:
            from trn_agent_boot.trn_boot import _ntff_profile_via_ctypes

            hook = _ntff_profile_via_ctypes("/opt/axon/libaxon_pjrt.so")
        except Exception:
            hook = None
        mod = types.ModuleType("antenv.axon_hooks")
        mod.get_axon_ntff_profile_hook = lambda: hook
        mod.set_axon_ntff_profile_hook = lambda h: None
        sys.modules["antenv.axon_hooks"] = mod

    # artifact upload needs S3 creds we don't have; keep artifacts local.
    import concourse.bass_utils as bu

    bu.upload_artifacts = lambda tmpdir: "local://" + tmpdir

    # This walrus build accepts at most ONE sync-wait per non-DMA instruction
    # ("Too many sync wait commands"). Hoist extra waits onto single-wait
    # NoOps emitted just before the instruction on the same engine.
    import orjson

    def _split_multiwaits(bir: bytes) -> bytes:
        m = orjson.loads(bir)
        ctr = 0
        changed = False
        for f in m["functions"]:
            for blk in f["blocks"]:
                newinsts = []
                for inst in blk["instructions"]:
                    si = inst.get("sync_info")
                    if si and len(si.get("on_wait", [])) > 1:
                        waits = si["on_wait"]
                        for w in waits[:-1]:
                            ctr += 1
                            newinsts.append(
                                {
                                    "debug": inst.get("debug", 0),
                                    "engine": inst["engine"],
                                    "ins": [],
                                    "outs": [],
                                    "name": f"{inst['name']}_sw{ctr}",
                                    "opcode": "NoOp",
                                    "sync_info": {
                                        "on_wait": [w],
                                        "on_update": [],
                                    },
                                }
                            )
                        si["on_wait"] = [waits[-1]]
                        changed = True
                    newinsts.append(inst)
                blk["instructions"] = newinsts
        return orjson.dumps(m) if changed else bir

    _orig_tjb = bass.Bass.to_json_bytes

    def _to_json_bytes_split(self):
        return _split_multiwaits(_orig_tjb(self))

    bass.Bass.to_json_bytes = _to_json_bytes_split


_install_workarounds()


# ---------------------------------------------------------------------------
# Host-side router — replicates the reference router on jax-CPU
# ---------------------------------------------------------------------------
def _route(x, Wr, br):
    """Return comb [T, E] fp32 combine weights (0 for unselected experts) and
    top_idx [T, K] int — computed exactly as the reference does, on CPU."""
    import jax
    import jax.numpy as jnp

    cpu = jax.devices("cpu")[0]
    with jax.default_device(cpu):
        xj = jnp.asarray(np.asarray(x))
        logits = jnp.einsum("bsd,de->bse", xj, jnp.asarray(np.asarray(Wr)))
        logits = logits + jnp.asarray(np.asarray(br))
        top_vals, top_idx = jax.lax.top_k(logits, TOP_K)
        top_w = jax.nn.softmax(top_vals, axis=-1)
        comb = jnp.sum(
            jax.nn.one_hot(top_idx, N_EXPERTS, dtype=xj.dtype) * top_w[..., None],
            axis=-2,
        )
        comb_np = np.asarray(comb).reshape(-1, N_EXPERTS)
        idx_np = np.asarray(top_idx).reshape(-1, TOP_K)
    return comb_np, idx_np


def _seg_blocks(s, maxb=512):
    """Column blocks for a segment of s tokens. PSUM caps N at 512; an even
    split minimizes the per-instruction overhead (a 32-col remainder chain
    costs ~15ns/matmul extra, ~12us kernel-wide)."""
    nblk = -(-s // maxb)
    base, rem = divmod(s, nblk)
    out = []
    t0 = 0
    for i in range(nblk):
        nb = base + (1 if i < rem else 0)
        out.append((t0, nb))
        t0 += nb
    return out


# ---------------------------------------------------------------------------
# Device program (two expert segments per core, SPMD)
# ---------------------------------------------------------------------------
_prog_cache = {}
import os as _os
CHAIN_X = _os.environ.get("CHAIN_X", "0") == "1"
WP_BUFS = int(_os.environ.get("WP_BUFS", "2"))
XC_ENG = _os.environ.get("XC_ENG", "gs")
P1_CAP = int(_os.environ.get("P1_CAP", "512"))
INTERLEAVE_UV = _os.environ.get("ILV", "0") == "1"
CHAIN_W12 = _os.environ.get("CHAIN_W12", "0") == "1"
SPLIT0 = _os.environ.get("SPLIT0", "0") == "1"


def _build_program(s1, s2):
    """Bass program over C = s1 + s2 token slots: segment A = [0, s1) runs
    expert "a" weights, segment B = [s1, s1+s2) expert "b" weights.

    DMA sources are laid out for long contiguous runs (the DMA fabric moves
    packets per contiguous run; sub-1KB runs cost ~42ns each and cap well
    below line rate, >=4KB runs reach it).  x arrives as per-block packed
    chunks (u|v) sized to the phase-1 column blocks, streamed just-in-time
    for the first m-iteration; later m-iterations re-read the same tiles.
      xc<i> [P, 2, KO, nb] bf16   (j, ko, t) = x/xv[t0+t, ko*128+p]
      w120 [P, 2, KO, P] bf16     m=0 segment-A w1|w2 tile, 4KB runs
      w12a/b [MF, P, 2, KO, P]    (m, p, j, ko, f) = Wj[ko*128+p, m*128+f]
      w3a/b [NQ, P, MF, QD]       (q, p, k, d) = W3[k*128+p, q*128+d]
      y [NQ, P, C] bf16           (q, d, t) output, transposed layout
    """
    C = s1 + s2
    segs = [(0, s1), (s1, s2)]
    # (seg, global offset, width) per phase-1 block == per x chunk
    chunks = []
    for si, (off, s) in enumerate(segs):
        for t0, nb in _seg_blocks(s, P1_CAP):
            chunks.append((si, off + t0, nb))

    nc = bass.Bass()
    xc = [
        nc.dram_tensor(f"xc{i}", [P, 2, KO, nb], BF16, kind="ExternalInput")
        for i, (_, _, nb) in enumerate(chunks)
    ]
    w120 = nc.dram_tensor("w120", [P, 2, KO, P], BF16, kind="ExternalInput")
    w12a = nc.dram_tensor("w12a", [MF, P, 2, KO, P], BF16, kind="ExternalInput")
    w12b = nc.dram_tensor("w12b", [MF, P, 2, KO, P], BF16, kind="ExternalInput")
    w3a = nc.dram_tensor("w3a", [NQ, P, MF, QD], BF16, kind="ExternalInput")
    w3b = nc.dram_tensor("w3b", [NQ, P, MF, QD], BF16, kind="ExternalInput")
    y = nc.dram_tensor("y", [NQ, P, C], BF16, kind="ExternalOutput")

    with tile.TileContext(nc) as tc:
        with (
            tc.tile_pool(name="persist", bufs=1) as persist,
            tc.tile_pool(name="wp", bufs=WP_BUFS) as wp,
            tc.tile_pool(name="w3p", bufs=2) as w3p,
            tc.tile_pool(name="sp", bufs=3) as sp,
            tc.tile_pool(name="yp", bufs=3) as yp,
            tc.tile_pool(name="psA", bufs=2, space="PSUM") as psA,
            tc.tile_pool(name="psB", bufs=2, space="PSUM") as psB,
            tc.tile_pool(name="psY", bufs=3, space="PSUM") as psY,
        ):
            h_sb = persist.tile([P, MF, C], BF16)
            w120_sb = persist.tile([P, 2, KO, P], BF16)
            xc_sb = []
            for i, (_, _, nb) in enumerate(chunks):
                t = persist.tile([P, 2, KO, nb], BF16, name=f"xc_sb{i}")
                xc_sb.append(t)

            # startup: w120 on sync; x chunks JIT on gpsimd/scalar.  The DMA
            # fabric round-robins packets with no priority, so chunk i+1's
            # descriptor is chained behind chunk i's arrival with a dummy
            # 8-element copy (WAR dep): the startup-critical first chunk gets
            # the full fabric, later chunks stream during m=0 compute.
            if SPLIT0:
                # fabric round-robins packets per descriptor: splitting the
                # startup-critical transfers across descriptors/engines
                # raises their bandwidth share
                nc.sync.dma_start(w120_sb[:, 0], w120[:, 0])
                nc.sync.dma_start(w120_sb[:, 1], w120[:, 1])
                nc.gpsimd.dma_start(xc_sb[0][:, 0], xc[0][:, 0])
                nc.scalar.dma_start(xc_sb[0][:, 1], xc[0][:, 1])
            else:
                nc.sync.dma_start(w120_sb[:], w120[:])
                nc.gpsimd.dma_start(xc_sb[0][:], xc[0][:])
            for i, t in list(enumerate(xc_sb))[1:]:
                if CHAIN_X:
                    nc.vector.tensor_copy(
                        t[:, 0, 0, 0:8], xc_sb[i - 1][:, 0, 0, 0:8]
                    )
                (nc.gpsimd if (XC_ENG == "g" or i % 2 == 0) else nc.scalar).dma_start(t[:], xc[i][:])

            w12_dr = {0: w12a, 1: w12b}

            iters = [(m, si) for m in range(MF) for si in range(2)]
            prio_it = []
            for m, si in iters:
                prio_it.append(tc.cur_priority)
                w12t = None
                for ci, (csi, off, nb) in enumerate(chunks):
                    if csi != si:
                        continue
                    if w12t is None:
                        if (m, si) == (0, 0):
                            w12t = w120_sb
                        else:
                            w12t = wp.tile(
                                [P, 2, KO, P], BF16, tag=f"w12t{si}", name="w12t"
                            )
                            if CHAIN_W12 and (m, si) in ((1, 0), (2, 0)):
                                nc.vector.tensor_copy(
                                    w12t[:, 0, 0, 0:8], xc_sb[0][:, 0, 0, 0:8]
                                )
                            nc.sync.dma_start(w12t[:], w12_dr[si][m])
                    tsl = slice(off, off + nb)
                    ps1_f = psA.tile([P, nb], F32, tag="ps1", name="ps1",
                                     padded_shape=[P, 512])
                    ps2_f = psB.tile([P, nb], F32, tag="ps2", name="ps2",
                                     padded_shape=[P, 512])
                    if INTERLEAVE_UV:
                        for ko in range(KO):
                            for j, ps in ((0, ps1_f), (1, ps2_f)):
                                nc.tensor.matmul(
                                    ps,
                                    w12t[:, j, ko],
                                    xc_sb[ci][:, j, ko],
                                    start=(ko == 0),
                                    stop=(ko == KO - 1),
                                )
                    else:
                        for ko in range(KO):
                            nc.tensor.matmul(
                                ps1_f,
                                w12t[:, 0, ko],
                                xc_sb[ci][:, 0, ko],
                                start=(ko == 0),
                                stop=(ko == KO - 1),
                            )
                        for ko in range(KO):
                            nc.tensor.matmul(
                                ps2_f,
                                w12t[:, 1, ko],
                                xc_sb[ci][:, 1, ko],
                                start=(ko == 0),
                                stop=(ko == KO - 1),
                            )
                    sil = sp.tile([P, nb], F32, tag="sil", name="sil",
                                  padded_shape=[P, 512])
                    nc.scalar.activation(
                        sil, ps1_f, mybir.ActivationFunctionType.Silu
                    )
                    nc.vector.tensor_mul(h_sb[:, m, tsl], sil, ps2_f)

            # --- phase 2: y[q] = h.T @ W3[:, q*128:(q+1)*128], h moving ---
            w3_dr = {0: w3a, 1: w3b}
            for q in range(NQ):
                w3t = {}
                # prefetch this q's W3 as if issued mid-phase-1
                prio_save = tc.cur_priority
                for si in range(2):
                    w3t[si] = w3p.tile(
                        [P, MF, QD], BF16, tag=f"w3t{si}", name=f"w3t{si}"
                    )
                    if q < 2:
                        # spread this q's W3 stream over the segment-B half
                        # so it does not starve the tail w12 loads
                        for j, kg in enumerate(range(0, MF, 8)):
                            tc.cur_priority = prio_it[
                                min(2 * (10 + 11 * q + 3 * j) + si, 2 * MF - 1)
                            ]
                            nc.sync.dma_start(
                                w3t[si][:, kg : kg + 8], w3_dr[si][q, :, kg : kg + 8]
                            )
                    else:
                        nc.sync.dma_start(w3t[si][:], w3_dr[si][q])
                tc.cur_priority = prio_save
                # biggest chains first, smallest last so the final store +
                # drain tail is as small as possible
                chains = []
                for si, (off, s) in enumerate(segs):
                    for t0, nb in _seg_blocks(s):
                        chains.append((si, off + t0, nb))
                chains.sort(key=lambda c: -c[2])
                for si, t0, nb in chains:
                    tsl = slice(t0, t0 + nb)
                    psy_f = psY.tile([P, nb], F32, tag="psy", name="psy",
                                     padded_shape=[P, 512])
                    for k in range(MF):
                        nc.tensor.matmul(
                            psy_f,
                            w3t[si][:, k],
                            h_sb[:, k, tsl],
                            start=(k == 0),
                            stop=(k == MF - 1),
                        )
                    ysb = yp.tile([P, nb], BF16, tag="ysb", name="ysb",
                                  padded_shape=[P, 512])
                    nc.vector.tensor_copy(ysb, psy_f)
                    nc.sync.dma_start(y[q, :, tsl], ysb)
    return nc


def _get_program(s1, s2):
    key = (s1, s2, CHAIN_X, WP_BUFS, XC_ENG, P1_CAP, INTERLEAVE_UV, CHAIN_W12, SPLIT0)
    if key not in _prog_cache:
        _prog_cache[key] = _build_program(s1, s2)
    return _prog_cache[key]


# ---------------------------------------------------------------------------
# Public entry point
# ---------------------------------------------------------------------------
def kernel(x, Wr, br, W1, b1, W2, b2, W3, b3):
    x = np.asarray(x)
    Wr = np.asarray(Wr)
    br = np.asarray(br)
    W1 = np.asarray(W1)
    b1 = np.asarray(b1)
    W2 = np.asarray(W2)
    b2 = np.asarray(b2)
    W3 = np.asarray(W3)
    b3 = np.asarray(b3)

    B, S, _ = x.shape
    T = B * S
    xf = np.ascontiguousarray(x.reshape(T, D_MODEL))

    if np.any(b1) or np.any(b2):
        raise NotImplementedError("nonzero b1/b2 not supported by this kernel")

    comb, top_idx = _route(x, Wr, br)

    # Dispatch: gather each expert's tokens (host all-to-all).
    sels = []
    for e in range(N_EXPERTS):
        sel = np.nonzero((top_idx == e).any(axis=1))[0]
        sels.append(sel)
    counts = np.array([len(s) for s in sels])
    order = np.argsort(-counts, kind="stable")
    big, small = order[:4], order[4:]
    def _ceil16(n):
        return -(-n // 16) * 16

    s1 = max(512, _ceil16(-(-int(counts[big].max()) // 2)))
    s2 = max(256, _ceil16(-(-int(counts[small].max()) // 2)))
    C = s1 + s2

    # weight shuffles into DMA-friendly bf16 layouts (see _build_program)
    bf16 = ml_dtypes.bfloat16
    w1d = W1.reshape(N_EXPERTS, KO, P, MF, P).transpose(0, 3, 2, 1, 4).astype(bf16)
    w2d = W2.reshape(N_EXPERTS, KO, P, MF, P).transpose(0, 3, 2, 1, 4).astype(bf16)
    # pack w1|w2 per m-slice: [E, MF, P, 2, KO, P], 4KB contiguous rows
    w12d = np.ascontiguousarray(np.stack([w1d, w2d], axis=3))
    w3d = np.ascontiguousarray(
        W3.reshape(N_EXPERTS, MF, P, NQ, QD).transpose(0, 3, 2, 1, 4).astype(bf16)
    )

    # core 2i / 2i+1 share big[i] in segment A and small[i] in segment B
    seg_tok = {}  # core -> [(expert, tokens, off, size)]
    in_maps = []
    for c in range(8):
        i, half = divmod(c, 2)
        eb, es = int(big[i]), int(small[i])
        tokA = sels[eb][half * s1 : (half + 1) * s1]
        tokB = sels[es][half * s2 : (half + 1) * s2]
        seg_tok[c] = [(eb, tokA, 0), (es, tokB, s1)]

        xu_c = np.zeros((P, KO, C), dtype=bf16)
        xv_c = np.zeros((P, KO, C), dtype=bf16)
        for e, toks, off in seg_tok[c]:
            n = len(toks)
            if not n:
                continue
            xs = xf[toks]
            xu_c[:, :, off : off + n] = (
                xs.astype(bf16).reshape(n, KO, P).transpose(2, 1, 0)
            )
            xv_c[:, :, off : off + n] = (
                (xs * comb[toks, e][:, None])
                .astype(bf16)
                .reshape(n, KO, P)
                .transpose(2, 1, 0)
            )
        im = {
            "w120": w12d[eb][0],
            "w12a": w12d[eb],
            "w12b": w12d[es],
            "w3a": w3d[eb],
            "w3b": w3d[es],
        }
        ci = 0
        for soff, s in ((0, s1), (s1, s2)):
            for t0, nb in _seg_blocks(s, P1_CAP):
                a, b = soff + t0, soff + t0 + nb
                im[f"xc{ci}"] = np.ascontiguousarray(
                    np.stack([xu_c[:, :, a:b], xv_c[:, :, a:b]], axis=1)
                )
                ci += 1
        in_maps.append(im)

    nc = _get_program(s1, s2)
    try:
        res = run_bass_kernel_spmd(nc, in_maps, core_ids=list(range(N_EXPERTS)))
    except Exception:
        # transient NRT/axon device hiccups have been observed; retry once
        import time as _time

        _time.sleep(5)
        res = run_bass_kernel_spmd(nc, in_maps, core_ids=list(range(N_EXPERTS)))

    # Combine: scatter-add expert outputs (softmax weights already folded in).
    out = np.zeros((T, D_MODEL), dtype=np.float32)
    for c in range(8):
        yc = np.asarray(res.results[c]["y"], dtype=np.float32)  # [NQ, P, C]
        yt = yc.transpose(2, 0, 1).reshape(C, D_MODEL)
        for e, toks, off in seg_tok[c]:
            n = len(toks)
            if n:
                out[toks] += yt[off : off + n]
    if np.any(b3):
        out += comb @ b3
    return out.reshape(B, S, D_MODEL)


# revision 25
# speedup vs baseline: 1.0333x; 1.0067x over previous
"""MoE feed-forward (8 experts, top-2 routing) on 8 Trainium2 NeuronCores.

Strategy (balanced expert parallelism, all-bf16):
  - Router runs on host with jax-CPU, replicating the reference's fp32 ops
    (einsum + top_k + softmax) so expert selection matches exactly.
  - Expert identity is pure data under SPMD: every core runs the same
    program over two fixed-size token segments (s1=544, s2=512 slots), and
    each core's in_map supplies whichever experts' weights its segments
    need.  The 4 most-loaded experts are split across two cores' segment-A
    slots, the 4 least-loaded across two cores' segment-B slots, so every
    core processes C = s1+s2 = 1056 token slots (vs 1088 + phase-2 padding
    for one-expert-per-core).
  - The top-2 softmax combine weight is folded into the W2-path activations
    on the host (y = (silu(x@W1) * ((comb*x)@W2)) @ W3 is linear in the
    W2-path input), so the device applies no per-token scaling at all and
    phase 2 is token-granular.
  - Phase 1: h = silu(x@W1) * (xv@W2), bf16 in / f32 PSUM / bf16 h.
  - Phase 2: y[dslice] = W3_slice.T @ h contraction over d_ff with h as the
    moving operand; y stored bf16, host does the scatter-add combine.
"""

import sys
import types

for _p in ("/opt/trn_rl_repo", "/root/.axon_site/_ro/trn_rl_repo"):
    if _p not in sys.path:
        sys.path.append(_p)

import numpy as np
import ml_dtypes

import concourse.bass as bass
import concourse.mybir as mybir
import concourse.tile as tile
from concourse.bass_utils import run_bass_kernel_spmd

D_MODEL = 1024
D_FF = 4096
N_EXPERTS = 8
TOP_K = 2
P = 128
KO = D_MODEL // P  # 8 k-tiles over d_model
MF = D_FF // P  # 32 slices over d_ff
NQ = D_MODEL // P  # 8 output d_model slices
QD = P

F32 = mybir.dt.float32
BF16 = mybir.dt.bfloat16


# ---------------------------------------------------------------------------
# Workarounds for this container's toolchain
# ---------------------------------------------------------------------------
def _install_workarounds():
    # walrus here rejects >1 sync-wait on the TileContext-final Drain; split
    # the waits across a chain of single-wait drains.
    def _drain_and_barrier_split(self, tick_clock, wait_clock):
        drain_inst = self.nc.sync.drain()
        wait_clock.add_sem_waits(
            drain_inst.ins, tile.ScopedClock({None: tick_clock.global_clock})
        )
        si = drain_inst.ins.sync_info
        waits = list(si.on_wait) if si is not None else []
        if len(waits) > 1:
            si.on_wait = [waits[0]]
            for w in waits[1:]:
                d2 = self.nc.sync.drain()
                d2.ins.sync_info = mybir.SyncInfo(on_wait=[w], on_update=[])
        self.nc.all_engine_barrier()
        popped = self.nc._tile_sem_poison_stack.pop()
        assert popped is self._sem_poison
        self.nc.clear_and_free_semaphores(list(self.sems.allocated().values()))
        self.nc.all_engine_barrier()

    tile.TileContext._drain_and_barrier = _drain_and_barrier_split

    # antenv.axon_hooks is absent on this image; register the NTFF profile
    # hook from trn_agent_boot so trace=True works (no-op for trace=False).
    if "antenv.axon_hooks" not in sys.modules:
        try:
            from trn_agent_boot.trn_boot import _ntff_profile_via_ctypes

            hook = _ntff_profile_via_ctypes("/opt/axon/libaxon_pjrt.so")
        except Exception:
            hook = None
        mod = types.ModuleType("antenv.axon_hooks")
        mod.get_axon_ntff_profile_hook = lambda: hook
        mod.set_axon_ntff_profile_hook = lambda h: None
        sys.modules["antenv.axon_hooks"] = mod

    # artifact upload needs S3 creds we don't have; keep artifacts local.
    import concourse.bass_utils as bu

    bu.upload_artifacts = lambda tmpdir: "local://" + tmpdir

    # This walrus build accepts at most ONE sync-wait per non-DMA instruction
    # ("Too many sync wait commands"). Hoist extra waits onto single-wait
    # NoOps emitted just before the instruction on the same engine.
    import orjson

    def _split_multiwaits(bir: bytes) -> bytes:
        m = orjson.loads(bir)
        ctr = 0
        changed = False
        for f in m["functions"]:
            for blk in f["blocks"]:
                newinsts = []
                for inst in blk["instructions"]:
                    si = inst.get("sync_info")
                    if si and len(si.get("on_wait", [])) > 1:
                        waits = si["on_wait"]
                        for w in waits[:-1]:
                            ctr += 1
                            newinsts.append(
                                {
                                    "debug": inst.get("debug", 0),
                                    "engine": inst["engine"],
                                    "ins": [],
                                    "outs": [],
                                    "name": f"{inst['name']}_sw{ctr}",
                                    "opcode": "NoOp",
                                    "sync_info": {
                                        "on_wait": [w],
                                        "on_update": [],
                                    },
                                }
                            )
                        si["on_wait"] = [waits[-1]]
                        changed = True
                    newinsts.append(inst)
                blk["instructions"] = newinsts
        return orjson.dumps(m) if changed else bir

    _orig_tjb = bass.Bass.to_json_bytes

    def _to_json_bytes_split(self):
        return _split_multiwaits(_orig_tjb(self))

    bass.Bass.to_json_bytes = _to_json_bytes_split


_install_workarounds()


# ---------------------------------------------------------------------------
# Host-side router — replicates the reference router on jax-CPU
# ---------------------------------------------------------------------------
def _route(x, Wr, br):
    """Return comb [T, E] fp32 combine weights (0 for unselected experts) and
    top_idx [T, K] int — computed exactly as the reference does, on CPU."""
    import jax
    import jax.numpy as jnp

    cpu = jax.devices("cpu")[0]
    with jax.default_device(cpu):
        xj = jnp.asarray(np.asarray(x))
        logits = jnp.einsum("bsd,de->bse", xj, jnp.asarray(np.asarray(Wr)))
        logits = logits + jnp.asarray(np.asarray(br))
        top_vals, top_idx = jax.lax.top_k(logits, TOP_K)
        top_w = jax.nn.softmax(top_vals, axis=-1)
        comb = jnp.sum(
            jax.nn.one_hot(top_idx, N_EXPERTS, dtype=xj.dtype) * top_w[..., None],
            axis=-2,
        )
        comb_np = np.asarray(comb).reshape(-1, N_EXPERTS)
        idx_np = np.asarray(top_idx).reshape(-1, TOP_K)
    return comb_np, idx_np


def _seg_blocks(s, maxb=512):
    """Column blocks for a segment of s tokens. PSUM caps N at 512; an even
    split minimizes the per-instruction overhead (a 32-col remainder chain
    costs ~15ns/matmul extra, ~12us kernel-wide)."""
    nblk = -(-s // maxb)
    base, rem = divmod(s, nblk)
    out = []
    t0 = 0
    for i in range(nblk):
        nb = base + (1 if i < rem else 0)
        out.append((t0, nb))
        t0 += nb
    return out


# ---------------------------------------------------------------------------
# Device program (two expert segments per core, SPMD)
# ---------------------------------------------------------------------------
_prog_cache = {}
import os as _os
CHAIN_X = _os.environ.get("CHAIN_X", "0") == "1"
WP_BUFS = int(_os.environ.get("WP_BUFS", "2"))
XC_ENG = _os.environ.get("XC_ENG", "gs")
P1_CAP = int(_os.environ.get("P1_CAP", "512"))
INTERLEAVE_UV = _os.environ.get("ILV", "0") == "1"
CHAIN_W12 = _os.environ.get("CHAIN_W12", "0") == "1"
SPLIT0 = _os.environ.get("SPLIT0", "0") == "1"
PSY_BUFS = int(_os.environ.get("PSY_BUFS", "3"))


def _build_program(s1, s2):
    """Bass program over C = s1 + s2 token slots: segment A = [0, s1) runs
    expert "a" weights, segment B = [s1, s1+s2) expert "b" weights.

    DMA sources are laid out for long contiguous runs (the DMA fabric moves
    packets per contiguous run; sub-1KB runs cost ~42ns each and cap well
    below line rate, >=4KB runs reach it).  x arrives as per-block packed
    chunks (u|v) sized to the phase-1 column blocks, streamed just-in-time
    for the first m-iteration; later m-iterations re-read the same tiles.
      xc<i> [P, 2, KO, nb] bf16   (j, ko, t) = x/xv[t0+t, ko*128+p]
      w120 [P, 2, KO, P] bf16     m=0 segment-A w1|w2 tile, 4KB runs
      w12a/b [MF, P, 2, KO, P]    (m, p, j, ko, f) = Wj[ko*128+p, m*128+f]
      w3a/b [NQ, P, MF, QD]       (q, p, k, d) = W3[k*128+p, q*128+d]
      y [NQ, P, C] bf16           (q, d, t) output, transposed layout
    """
    C = s1 + s2
    segs = [(0, s1), (s1, s2)]
    # (seg, global offset, width) per phase-1 block == per x chunk
    chunks = []
    for si, (off, s) in enumerate(segs):
        for t0, nb in _seg_blocks(s, P1_CAP):
            chunks.append((si, off + t0, nb))

    nc = bass.Bass()
    xc = [
        nc.dram_tensor(f"xc{i}", [P, 2, KO, nb], BF16, kind="ExternalInput")
        for i, (_, _, nb) in enumerate(chunks)
    ]
    w120 = nc.dram_tensor("w120", [P, 2, KO, P], BF16, kind="ExternalInput")
    w12a = nc.dram_tensor("w12a", [MF, P, 2, KO, P], BF16, kind="ExternalInput")
    w12b = nc.dram_tensor("w12b", [MF, P, 2, KO, P], BF16, kind="ExternalInput")
    w3a = nc.dram_tensor("w3a", [NQ, P, MF, QD], BF16, kind="ExternalInput")
    w3b = nc.dram_tensor("w3b", [NQ, P, MF, QD], BF16, kind="ExternalInput")
    y = nc.dram_tensor("y", [NQ, P, C], BF16, kind="ExternalOutput")

    with tile.TileContext(nc) as tc:
        with (
            tc.tile_pool(name="persist", bufs=1) as persist,
            tc.tile_pool(name="wp", bufs=WP_BUFS) as wp,
            tc.tile_pool(name="w3p", bufs=2) as w3p,
            tc.tile_pool(name="sp", bufs=3) as sp,
            tc.tile_pool(name="yp", bufs=3) as yp,
            tc.tile_pool(name="psA", bufs=2, space="PSUM") as psA,
            tc.tile_pool(name="psB", bufs=2, space="PSUM") as psB,
            tc.tile_pool(name="psY", bufs=PSY_BUFS, space="PSUM") as psY,
        ):
            h_sb = persist.tile([P, MF, C], BF16)
            w120_sb = persist.tile([P, 2, KO, P], BF16)
            xc_sb = []
            for i, (_, _, nb) in enumerate(chunks):
                t = persist.tile([P, 2, KO, nb], BF16, name=f"xc_sb{i}")
                xc_sb.append(t)

            # startup: w120 on sync; x chunks JIT on gpsimd/scalar.  The DMA
            # fabric round-robins packets with no priority, so chunk i+1's
            # descriptor is chained behind chunk i's arrival with a dummy
            # 8-element copy (WAR dep): the startup-critical first chunk gets
            # the full fabric, later chunks stream during m=0 compute.
            if SPLIT0:
                # fabric round-robins packets per descriptor: splitting the
                # startup-critical transfers across descriptors/engines
                # raises their bandwidth share
                nc.sync.dma_start(w120_sb[:, 0], w120[:, 0])
                nc.sync.dma_start(w120_sb[:, 1], w120[:, 1])
                nc.gpsimd.dma_start(xc_sb[0][:, 0], xc[0][:, 0])
                nc.scalar.dma_start(xc_sb[0][:, 1], xc[0][:, 1])
            else:
                nc.sync.dma_start(w120_sb[:], w120[:])
                nc.gpsimd.dma_start(xc_sb[0][:], xc[0][:])
            for i, t in list(enumerate(xc_sb))[1:]:
                if CHAIN_X:
                    nc.vector.tensor_copy(
                        t[:, 0, 0, 0:8], xc_sb[i - 1][:, 0, 0, 0:8]
                    )
                (nc.gpsimd if (XC_ENG == "g" or i % 2 == 0) else nc.scalar).dma_start(t[:], xc[i][:])

            w12_dr = {0: w12a, 1: w12b}

            iters = [(m, si) for m in range(MF) for si in range(2)]
            prio_it = []
            for m, si in iters:
                prio_it.append(tc.cur_priority)
                w12t = None
                for ci, (csi, off, nb) in enumerate(chunks):
                    if csi != si:
                        continue
                    if w12t is None:
                        if (m, si) == (0, 0):
                            w12t = w120_sb
                        else:
                            w12t = wp.tile(
                                [P, 2, KO, P], BF16, tag=f"w12t{si}", name="w12t"
                            )
                            if CHAIN_W12 and (m, si) in ((1, 0), (2, 0)):
                                nc.vector.tensor_copy(
                                    w12t[:, 0, 0, 0:8], xc_sb[0][:, 0, 0, 0:8]
                                )
                            nc.sync.dma_start(w12t[:], w12_dr[si][m])
                    tsl = slice(off, off + nb)
                    ps1_f = psA.tile([P, nb], F32, tag="ps1", name="ps1",
                                     padded_shape=[P, 512])
                    ps2_f = psB.tile([P, nb], F32, tag="ps2", name="ps2",
                                     padded_shape=[P, 512])
                    if INTERLEAVE_UV:
                        for ko in range(KO):
                            for j, ps in ((0, ps1_f), (1, ps2_f)):
                                nc.tensor.matmul(
                                    ps,
                                    w12t[:, j, ko],
                                    xc_sb[ci][:, j, ko],
                                    start=(ko == 0),
                                    stop=(ko == KO - 1),
                                )
                    else:
                        for ko in range(KO):
                            nc.tensor.matmul(
                                ps1_f,
                                w12t[:, 0, ko],
                                xc_sb[ci][:, 0, ko],
                                start=(ko == 0),
                                stop=(ko == KO - 1),
                            )
                        for ko in range(KO):
                            nc.tensor.matmul(
                                ps2_f,
                                w12t[:, 1, ko],
                                xc_sb[ci][:, 1, ko],
                                start=(ko == 0),
                                stop=(ko == KO - 1),
                            )
                    sil = sp.tile([P, nb], F32, tag="sil", name="sil",
                                  padded_shape=[P, 512])
                    nc.scalar.activation(
                        sil, ps1_f, mybir.ActivationFunctionType.Silu
                    )
                    nc.vector.tensor_mul(h_sb[:, m, tsl], sil, ps2_f)

            # --- phase 2: y[q] = h.T @ W3[:, q*128:(q+1)*128], h moving ---
            w3_dr = {0: w3a, 1: w3b}
            for q in range(NQ):
                w3t = {}
                # prefetch this q's W3 as if issued mid-phase-1
                prio_save = tc.cur_priority
                for si in range(2):
                    w3t[si] = w3p.tile(
                        [P, MF, QD], BF16, tag=f"w3t{si}", name=f"w3t{si}"
                    )
                    if q < 2:
                        # spread this q's W3 stream over the segment-B half
                        # so it does not starve the tail w12 loads
                        for j, kg in enumerate(range(0, MF, 8)):
                            tc.cur_priority = prio_it[
                                min(2 * (10 + 11 * q + 3 * j) + si, 2 * MF - 1)
                            ]
                            nc.sync.dma_start(
                                w3t[si][:, kg : kg + 8], w3_dr[si][q, :, kg : kg + 8]
                            )
                    else:
                        nc.sync.dma_start(w3t[si][:], w3_dr[si][q])
                tc.cur_priority = prio_save
                # biggest chains first, smallest last so the final store +
                # drain tail is as small as possible
                chains = []
                for si, (off, s) in enumerate(segs):
                    for t0, nb in _seg_blocks(s):
                        chains.append((si, off + t0, nb))
                chains.sort(key=lambda c: -c[2])
                for si, t0, nb in chains:
                    tsl = slice(t0, t0 + nb)
                    psy_f = psY.tile([P, nb], F32, tag="psy", name="psy",
                                     padded_shape=[P, 512])
                    for k in range(MF):
                        nc.tensor.matmul(
                            psy_f,
                            w3t[si][:, k],
                            h_sb[:, k, tsl],
                            start=(k == 0),
                            stop=(k == MF - 1),
                        )
                    ysb = yp.tile([P, nb], BF16, tag="ysb", name="ysb",
                                  padded_shape=[P, 512])
                    nc.vector.tensor_copy(ysb, psy_f)
                    nc.sync.dma_start(y[q, :, tsl], ysb)
    return nc


def _get_program(s1, s2):
    key = (s1, s2, CHAIN_X, WP_BUFS, XC_ENG, P1_CAP, INTERLEAVE_UV, CHAIN_W12, SPLIT0, PSY_BUFS)
    if key not in _prog_cache:
        _prog_cache[key] = _build_program(s1, s2)
    return _prog_cache[key]


# ---------------------------------------------------------------------------
# Public entry point
# ---------------------------------------------------------------------------
def kernel(x, Wr, br, W1, b1, W2, b2, W3, b3):
    x = np.asarray(x)
    Wr = np.asarray(Wr)
    br = np.asarray(br)
    W1 = np.asarray(W1)
    b1 = np.asarray(b1)
    W2 = np.asarray(W2)
    b2 = np.asarray(b2)
    W3 = np.asarray(W3)
    b3 = np.asarray(b3)

    B, S, _ = x.shape
    T = B * S
    xf = np.ascontiguousarray(x.reshape(T, D_MODEL))

    if np.any(b1) or np.any(b2):
        raise NotImplementedError("nonzero b1/b2 not supported by this kernel")

    comb, top_idx = _route(x, Wr, br)

    # Dispatch: gather each expert's tokens (host all-to-all).
    sels = []
    for e in range(N_EXPERTS):
        sel = np.nonzero((top_idx == e).any(axis=1))[0]
        sels.append(sel)
    counts = np.array([len(s) for s in sels])
    order = np.argsort(-counts, kind="stable")
    big, small = order[:4], order[4:]
    def _ceil16(n):
        return -(-n // 16) * 16

    s1 = max(512, _ceil16(-(-int(counts[big].max()) // 2)))
    s2 = max(256, _ceil16(-(-int(counts[small].max()) // 2)))
    C = s1 + s2

    # weight shuffles into DMA-friendly bf16 layouts (see _build_program)
    bf16 = ml_dtypes.bfloat16
    w1d = W1.reshape(N_EXPERTS, KO, P, MF, P).transpose(0, 3, 2, 1, 4).astype(bf16)
    w2d = W2.reshape(N_EXPERTS, KO, P, MF, P).transpose(0, 3, 2, 1, 4).astype(bf16)
    # pack w1|w2 per m-slice: [E, MF, P, 2, KO, P], 4KB contiguous rows
    w12d = np.ascontiguousarray(np.stack([w1d, w2d], axis=3))
    w3d = np.ascontiguousarray(
        W3.reshape(N_EXPERTS, MF, P, NQ, QD).transpose(0, 3, 2, 1, 4).astype(bf16)
    )

    # core 2i / 2i+1 share big[i] in segment A and small[i] in segment B
    seg_tok = {}  # core -> [(expert, tokens, off, size)]
    in_maps = []
    for c in range(8):
        i, half = divmod(c, 2)
        eb, es = int(big[i]), int(small[i])
        tokA = sels[eb][half * s1 : (half + 1) * s1]
        tokB = sels[es][half * s2 : (half + 1) * s2]
        seg_tok[c] = [(eb, tokA, 0), (es, tokB, s1)]

        xu_c = np.zeros((P, KO, C), dtype=bf16)
        xv_c = np.zeros((P, KO, C), dtype=bf16)
        for e, toks, off in seg_tok[c]:
            n = len(toks)
            if not n:
                continue
            xs = xf[toks]
            xu_c[:, :, off : off + n] = (
                xs.astype(bf16).reshape(n, KO, P).transpose(2, 1, 0)
            )
            xv_c[:, :, off : off + n] = (
                (xs * comb[toks, e][:, None])
                .astype(bf16)
                .reshape(n, KO, P)
                .transpose(2, 1, 0)
            )
        im = {
            "w120": w12d[eb][0],
            "w12a": w12d[eb],
            "w12b": w12d[es],
            "w3a": w3d[eb],
            "w3b": w3d[es],
        }
        ci = 0
        for soff, s in ((0, s1), (s1, s2)):
            for t0, nb in _seg_blocks(s, P1_CAP):
                a, b = soff + t0, soff + t0 + nb
                im[f"xc{ci}"] = np.ascontiguousarray(
                    np.stack([xu_c[:, :, a:b], xv_c[:, :, a:b]], axis=1)
                )
                ci += 1
        in_maps.append(im)

    nc = _get_program(s1, s2)
    try:
        res = run_bass_kernel_spmd(nc, in_maps, core_ids=list(range(N_EXPERTS)))
    except Exception:
        # transient NRT/axon device hiccups have been observed; retry once
        import time as _time

        _time.sleep(5)
        res = run_bass_kernel_spmd(nc, in_maps, core_ids=list(range(N_EXPERTS)))

    # Combine: scatter-add expert outputs (softmax weights already folded in).
    out = np.zeros((T, D_MODEL), dtype=np.float32)
    for c in range(8):
        yc = np.asarray(res.results[c]["y"], dtype=np.float32)  # [NQ, P, C]
        yt = yc.transpose(2, 0, 1).reshape(C, D_MODEL)
        for e, toks, off in seg_tok[c]:
            n = len(toks)
            if n:
                out[toks] += yt[off : off + n]
    if np.any(b3):
        out += comb @ b3
    return out.reshape(B, S, D_MODEL)
